# revision 26
# baseline (speedup 1.0000x reference)
"""Trainium2 Bass kernel for the CAM-drop attention module.

Reference computation (per sample n):
    cams  = relu(W @ x[n])            # W: [C=64, Cin=1024], x[n]: [Cin, H*W]
    thr_k = gama * max_hw(cams[k])    # per-channel spatial max
    drop  = where(cams > thr, 0, cams)
    out[n] = x[n] * mean_k(drop)      # broadcast over Cin

Data-parallel over the batch: 32 samples sharded 4-per-core across 8
NeuronCores; fc_weights / gama replicated. No cross-core communication.

The problem is HBM-bound, so x is pre-cast to bf16 on the host and loaded
as bf16, and the output is stored as bf16 and widened to f32 on the host
(halves both HBM streams; rel err stays ~7e-3, well under the 2e-2 gate).
Matmuls accumulate bf16 into f32 PSUM; the channel mean is bf16.

Per-core pipeline (samples unrolled):
  - x[n] streamed as 8 bf16 tiles [128, 3136] into a rotating SBUF pool
  - cams accumulated in f32 PSUM over the 8 Cin tiles (7 banks of 448)
  - per-bank relu (ACT) + partial spatial max (DVE) chase the matmul tail
  - threshold, in-place drop-mask (DVE)
  - channel mean via a bf16 [64->128] ones/64 matmul (fused broadcast+scale),
    copied per-bank PSUM->SBUF on ACT
  - out tile (bf16) = xb tile * mean_bf16 (DVE 2x mode), DMA out; first
    product chunked per bank to chase the copies, its store on the scalar
    HWDGE ring; host widens the bf16 output back to f32
"""

import numpy as np

# Problem shape (hardcoded per harness contract).
N, CIN, H, W = 32, 1024, 56, 56
C = 64
HW = H * W          # 3136
NCORES = 8
NS = N // NCORES    # 4 samples per core
P = 128             # SBUF partitions
NT = CIN // P       # 8 Cin tiles
NCH = 7             # spatial chunks per sample
CH = HW // NCH      # 448 (fits one PSUM bank)
BANK = 512          # PSUM bank stride in f32 elements
NBBUF = 20          # rotating bf16 x-tile slots (0.784 MB each)
NOBUF = 10          # rotating bf16 out-tile slots (0.784 MB each)

_CACHE = {}


def _build_nc():
    from concourse import bacc, bass, tile
    from concourse import mybir

    f32 = mybir.dt.float32
    bf16 = mybir.dt.bfloat16
    alu = mybir.AluOpType

    nc = bacc.Bacc("TRN2", target_bir_lowering=False, debug=False)
    x_ext = nc.declare_dram_parameter("x", [NS, CIN, HW], bf16, isOutput=False)
    wT_ext = nc.declare_dram_parameter("fc_weights", [CIN, C], bf16, isOutput=False)
    g_ext = nc.declare_dram_parameter("gama", [C, 1], f32, isOutput=False)
    out_ext = nc.declare_dram_parameter("out", [NS, CIN, HW], bf16, isOutput=True)

    with tile.TileContext(nc) as tc:
        with (
            tc.tile_pool(name="consts", bufs=1) as constp,
            tc.tile_pool(name="xbp", bufs=NBBUF) as xbp,
            tc.tile_pool(name="outp", bufs=NOBUF) as outp,
            tc.tile_pool(name="stats", bufs=2) as statp,
            tc.tile_pool(name="camsb", bufs=1) as camp,
            tc.tile_pool(name="meanp", bufs=1) as meanp,
            tc.tile_pool(name="psum", bufs=1, space=bass.MemorySpace.PSUM) as psump,
        ):
            w_sb = constp.tile([P, NT, C], bf16)
            for t in range(NT):
                nc.sync.dma_start(out=w_sb[:, t, :], in_=wT_ext[t * P:(t + 1) * P, :])
            g_sb = constp.tile([C, 1], f32)
            nc.sync.dma_start(out=g_sb[:], in_=g_ext[:])
            ones_sb = constp.tile([C, P], bf16)
            nc.vector.memset(ones_sb[:], 1.0 / C)

            # PE clock warm-up: the HAM gate holds the PE at half clock until
            # ~4us of sustained matmul activity. Garbage matmuls into a spare
            # PSUM bank (never read; DCE keeps unread matmuls) warm it up
            # during the initial load-only DMA phase.
            warm_ps = psump.tile([C, BANK], f32, name="warm_ps", tag="warm")
            w_flat = w_sb[:].rearrange("p a b -> p (a b)")
            for _ in range(15):
                nc.tensor.matmul(
                    warm_ps[:, :], w_sb[:, 0, :], w_flat[:, 0:BANK],
                    start=True, stop=True,
                )

            for n in range(NS):
                xbs = []
                for t in range(NT):
                    xb = xbp.tile([P, HW], bf16, name=f"xb_{n}_{t}", tag="xb")
                    nc.sync.dma_start(out=xb[:], in_=x_ext[n, t * P:(t + 1) * P, :])
                    xbs.append(xb)

                cams = psump.tile([C, NCH, BANK], f32, name=f"cams_{n}", tag="ps")
                crelu = camp.tile([C, NCH, CH], bf16, name=f"crelu_{n}", tag="crelu")
                # All matmuls first, then the per-bank relus and partial
                # maxes: interleaving readers with the (t == NT-1) matmuls
                # makes Tile serialize each matmul behind the previous bank's
                # readers (tile-granularity WAR), stretching the tail ~4x.
                for t in range(NT):
                    for s in range(NCH):
                        nc.tensor.matmul(
                            cams[:, s, 0:CH],
                            w_sb[:, t, :],
                            xbs[t][:, s * CH:(s + 1) * CH],
                            start=(t == 0),
                            stop=(t == NT - 1),
                        )
                for s0, s1 in ((0, 4), (4, NCH)):
                    nc.scalar.activation(
                        crelu[:, s0:s1, :], cams[:, s0:s1, 0:CH],
                        mybir.ActivationFunctionType.Relu,
                    )
                cmax = statp.tile([C, 1], f32, name=f"cmax_{n}", tag="cmax")
                nc.vector.tensor_reduce(
                    cmax[:], cams[:, :, 0:CH], axis=mybir.AxisListType.XY,
                    op=alu.max,
                )
                # thr = relu(cmax) * gama  (cams post-relu; max may be <0 pre-relu)
                thr = statp.tile([C, 1], f32, name=f"thr_{n}", tag="thr")
                nc.vector.tensor_scalar(
                    thr[:], cmax[:], 0.0, g_sb[:], op0=alu.max, op1=alu.mult
                )

                # drop = crelu * (crelu <= thr), in place (comparing post-relu
                # values against thr >= 0 matches the reference's pre-relu
                # compare). Then the channel mean, broadcast to all 128
                # partitions via a ones/64 matmul. Emit all masks, then all
                # matmuls, then all PSUM->SBUF copies: interleaving them makes
                # Tile serialize each matmul behind the previous bank's copy
                # (WAR on the shared mean tile).
                mean_ps = psump.tile([P, NCH, BANK], f32, name=f"meanps_{n}", tag="ps")
                mean_sb = meanp.tile([P, HW], bf16, name=f"mean_{n}", tag="mean")
                mean_sb3 = mean_sb[:].rearrange("p (a b) -> p a b", a=NCH)
                for s0, s1 in ((0, 4), (4, NCH)):
                    nc.vector.scalar_tensor_tensor(
                        crelu[:, s0:s1, :], crelu[:, s0:s1, :], thr[:],
                        crelu[:, s0:s1, :], op0=alu.is_le, op1=alu.mult,
                    )
                for s in range(NCH):
                    nc.tensor.matmul(
                        mean_ps[:, s, 0:CH], ones_sb[:], crelu[:, s, :],
                        start=True, stop=True,
                    )
                for s in range(NCH):
                    nc.scalar.copy(mean_sb3[:, s, :], mean_ps[:, s, 0:CH])

                # First product is chunked per bank so it chases the ACT
                # copies instead of waiting for the full mean tile, and its
                # store goes out on the (idle) scalar HWDGE ring — both pull
                # the store stream start a few us earlier at each boundary.
                outs = [
                    outp.tile([P, HW], bf16, name=f"o_{n}_{t}", tag="ot")
                    for t in range(NT)
                ]
                o0 = outs[0][:].rearrange("p (a b) -> p a b", a=NCH)
                xb0 = xbs[0][:].rearrange("p (a b) -> p a b", a=NCH)
                for s in range(NCH):
                    nc.vector.tensor_mul(
                        o0[:, s, :], xb0[:, s, :], mean_sb3[:, s, :]
                    )
                nc.scalar.dma_start(out=out_ext[n, 0:P, :], in_=outs[0][:])
                for t in range(1, NT):
                    nc.vector.tensor_mul(outs[t][:], xbs[t][:], mean_sb[:])
                    nc.gpsimd.dma_start(
                        out=out_ext[n, t * P:(t + 1) * P, :], in_=outs[t][:]
                    )
    nc.compile()
    return nc


def _get_nc():
    if "nc" not in _CACHE:
        _CACHE["nc"] = _build_nc()
    return _CACHE["nc"]


def _make_in_maps(x, fc_weights, gama):
    from concourse import mybir

    bf16_np = mybir.dt.np(mybir.dt.bfloat16)
    x = np.asarray(x, dtype=np.float32)
    wT = np.ascontiguousarray(
        np.asarray(fc_weights, dtype=np.float32).reshape(C, CIN).T
    ).astype(bf16_np)
    g64 = np.ascontiguousarray(
        np.broadcast_to(np.asarray(gama, dtype=np.float32).reshape(1, 1), (C, 1))
    )
    return [
        {
            "x": np.ascontiguousarray(
                x[i * NS:(i + 1) * NS].reshape(NS, CIN, HW)
            ).astype(bf16_np),
            "fc_weights": wT,
            "gama": g64,
        }
        for i in range(NCORES)
    ]


def kernel(x: np.ndarray, fc_weights: np.ndarray, gama: np.ndarray) -> np.ndarray:
    from concourse.bass_utils import run_bass_kernel_spmd

    nc = _get_nc()
    in_maps = _make_in_maps(x, fc_weights, gama)
    res = run_bass_kernel_spmd(nc, in_maps, core_ids=list(range(NCORES)))
    out = np.concatenate(
        [
            res.results[i]["out"].astype(np.float32).reshape(NS, CIN, H, W)
            for i in range(NCORES)
        ],
        axis=0,
    )
    return out


# revision 31
# speedup vs baseline: 1.1133x; 1.1133x over previous
"""Trainium2 Bass kernel for the CAM-drop attention module.

Reference computation (per sample n):
    cams  = relu(W @ x[n])            # W: [C=64, Cin=1024], x[n]: [Cin, H*W]
    thr_k = gama * max_hw(cams[k])    # per-channel spatial max
    drop  = where(cams > thr, 0, cams)
    out[n] = x[n] * mean_k(drop)      # broadcast over Cin

Data-parallel over the batch: 32 samples sharded 4-per-core across 8
NeuronCores; fc_weights / gama replicated. No cross-core communication.

The problem is HBM-bound, so x is pre-cast to bf16 on the host and loaded
as bf16, and the output is stored as bf16 and widened to f32 on the host
(halves both HBM streams; rel err stays ~7e-3, well under the 2e-2 gate).
Matmuls accumulate bf16 into f32 PSUM; the channel mean is bf16.

Per-core pipeline (samples unrolled):
  - x[n] streamed as 8 bf16 tiles [128, 3136] into a rotating SBUF pool
  - cams accumulated in f32 PSUM over the 8 Cin tiles (7 banks of 448)
  - per-bank relu (ACT) + partial spatial max (DVE) chase the matmul tail
  - threshold, in-place drop-mask (DVE)
  - channel mean via a bf16 [64->128] ones/64 matmul (fused broadcast+scale),
    copied per-bank PSUM->SBUF on ACT
  - out tile (bf16) = xb tile * mean_bf16 (DVE 2x mode), DMA out; first
    product chunked per bank to chase the copies, its store on the scalar
    HWDGE ring; host widens the bf16 output back to f32
"""

import numpy as np

# Problem shape (hardcoded per harness contract).
N, CIN, H, W = 32, 1024, 56, 56
C = 64
HW = H * W          # 3136
NCORES = 8
NS = N // NCORES    # 4 samples per core
P = 128             # SBUF partitions
NT = CIN // P       # 8 Cin tiles
NCH = 7             # spatial chunks per sample
CH = HW // NCH      # 448 (fits one PSUM bank)
BANK = 512          # PSUM bank stride in f32 elements
NBBUF = 24          # rotating bf16 x-tile slots (0.784 MB each)
NOBUF = 7           # rotating bf16 out-tile slots (0.784 MB each)

_CACHE = {}


def _build_nc():
    from concourse import bacc, bass, tile
    from concourse import mybir

    f32 = mybir.dt.float32
    bf16 = mybir.dt.bfloat16
    alu = mybir.AluOpType

    nc = bacc.Bacc("TRN2", target_bir_lowering=False, debug=False)
    x_ext = nc.declare_dram_parameter("x", [NS, CIN, HW], bf16, isOutput=False)
    wT_ext = nc.declare_dram_parameter("fc_weights", [CIN, C], bf16, isOutput=False)
    g_ext = nc.declare_dram_parameter("gama", [C, 1], f32, isOutput=False)
    out_ext = nc.declare_dram_parameter("out", [NS, CIN, HW], bf16, isOutput=True)

    with tile.TileContext(nc) as tc:
        with (
            tc.tile_pool(name="consts", bufs=1) as constp,
            tc.tile_pool(name="xbp", bufs=NBBUF) as xbp,
            tc.tile_pool(name="outp", bufs=NOBUF) as outp,
            tc.tile_pool(name="stats", bufs=2) as statp,
            tc.tile_pool(name="camsb", bufs=1) as camp,
            tc.tile_pool(name="meanp", bufs=1) as meanp,
            tc.tile_pool(name="psum", bufs=1, space=bass.MemorySpace.PSUM) as psump,
        ):
            w_sb = constp.tile([P, NT, C], bf16)
            for t in range(NT):
                nc.sync.dma_start(out=w_sb[:, t, :], in_=wT_ext[t * P:(t + 1) * P, :])
            g_sb = constp.tile([C, 1], f32)
            nc.sync.dma_start(out=g_sb[:], in_=g_ext[:])
            ones_sb = constp.tile([C, P], bf16)
            nc.vector.memset(ones_sb[:], 1.0 / C)

            # PE clock warm-up: the HAM gate holds the PE at half clock until
            # ~4us of sustained matmul activity. Garbage matmuls into a spare
            # PSUM bank (never read; DCE keeps unread matmuls) warm it up
            # during the initial load-only DMA phase.
            warm_ps = psump.tile([C, BANK], f32, name="warm_ps", tag="warm")
            w_flat = w_sb[:].rearrange("p a b -> p (a b)")
            for _ in range(15):
                nc.tensor.matmul(
                    warm_ps[:, :], w_sb[:, 0, :], w_flat[:, 0:BANK],
                    start=True, stop=True,
                )

            for n in range(NS):
                xbs = []
                for t in range(NT):
                    xb = xbp.tile([P, HW], bf16, name=f"xb_{n}_{t}", tag="xb")
                    nc.sync.dma_start(out=xb[:], in_=x_ext[n, t * P:(t + 1) * P, :])
                    xbs.append(xb)

                cams = psump.tile([C, NCH, BANK], f32, name=f"cams_{n}", tag="ps")
                crelu = camp.tile([C, NCH, CH], bf16, name=f"crelu_{n}", tag="crelu")
                # All matmuls first, then the per-bank relus and partial
                # maxes: interleaving readers with the (t == NT-1) matmuls
                # makes Tile serialize each matmul behind the previous bank's
                # readers (tile-granularity WAR), stretching the tail ~4x.
                for t in range(NT):
                    for s in range(NCH):
                        nc.tensor.matmul(
                            cams[:, s, 0:CH],
                            w_sb[:, t, :],
                            xbs[t][:, s * CH:(s + 1) * CH],
                            start=(t == 0),
                            stop=(t == NT - 1),
                        )
                for s0, s1 in ((0, 4), (4, NCH)):
                    nc.scalar.activation(
                        crelu[:, s0:s1, :], cams[:, s0:s1, 0:CH],
                        mybir.ActivationFunctionType.Relu,
                    )
                # Spatial max from the SBUF relu copy (not PSUM) so the cams
                # banks' last reader is the relu — the PSUM slot turns over to
                # the mean matmuls sooner. max(crelu) == relu(max(cams)), so
                # thr = max(crelu) * gama directly.
                cmax = statp.tile([C, 1], f32, name=f"cmax_{n}", tag="cmax")
                nc.vector.tensor_reduce(
                    cmax[:], crelu[:, :, :], axis=mybir.AxisListType.XY,
                    op=alu.max,
                )
                thr = statp.tile([C, 1], f32, name=f"thr_{n}", tag="thr")
                nc.vector.tensor_scalar(
                    thr[:], cmax[:], g_sb[:], None, op0=alu.mult
                )

                # drop = crelu * (crelu <= thr), in place (comparing post-relu
                # values against thr >= 0 matches the reference's pre-relu
                # compare). Then the channel mean, broadcast to all 128
                # partitions via a ones/64 matmul. Emit all masks, then all
                # matmuls, then all PSUM->SBUF copies: interleaving them makes
                # Tile serialize each matmul behind the previous bank's copy
                # (WAR on the shared mean tile).
                mean_ps = psump.tile([P, NCH, BANK], f32, name=f"meanps_{n}", tag="ps")
                mean_sb = meanp.tile([P, HW], bf16, name=f"mean_{n}", tag="mean")
                mean_sb3 = mean_sb[:].rearrange("p (a b) -> p a b", a=NCH)
                for s0, s1 in ((0, 4), (4, NCH)):
                    nc.vector.scalar_tensor_tensor(
                        crelu[:, s0:s1, :], crelu[:, s0:s1, :], thr[:],
                        crelu[:, s0:s1, :], op0=alu.is_le, op1=alu.mult,
                    )
                for s in range(NCH):
                    nc.tensor.matmul(
                        mean_ps[:, s, 0:CH], ones_sb[:], crelu[:, s, :],
                        start=True, stop=True,
                    )
                for s in range(NCH):
                    nc.scalar.copy(mean_sb3[:, s, :], mean_ps[:, s, 0:CH])

                # First product is chunked per bank so it chases the ACT
                # copies instead of waiting for the full mean tile, and its
                # store goes out on the (idle) scalar HWDGE ring — both pull
                # the store stream start a few us earlier at each boundary.
                outs = [
                    outp.tile([P, HW], bf16, name=f"o_{n}_{t}", tag="ot")
                    for t in range(NT)
                ]
                o0 = outs[0][:].rearrange("p (a b) -> p a b", a=NCH)
                xb0 = xbs[0][:].rearrange("p (a b) -> p a b", a=NCH)
                for s in range(NCH):
                    nc.vector.tensor_mul(
                        o0[:, s, :], xb0[:, s, :], mean_sb3[:, s, :]
                    )
                nc.scalar.dma_start(out=out_ext[n, 0:P, :], in_=outs[0][:])
                for t in range(1, NT):
                    nc.vector.tensor_mul(outs[t][:], xbs[t][:], mean_sb[:])
                    nc.gpsimd.dma_start(
                        out=out_ext[n, t * P:(t + 1) * P, :], in_=outs[t][:]
                    )
    nc.compile()
    return nc


def _get_nc():
    if "nc" not in _CACHE:
        _CACHE["nc"] = _build_nc()
    return _CACHE["nc"]


def _make_in_maps(x, fc_weights, gama):
    from concourse import mybir

    bf16_np = mybir.dt.np(mybir.dt.bfloat16)
    x = np.asarray(x, dtype=np.float32)
    wT = np.ascontiguousarray(
        np.asarray(fc_weights, dtype=np.float32).reshape(C, CIN).T
    ).astype(bf16_np)
    g64 = np.ascontiguousarray(
        np.broadcast_to(np.asarray(gama, dtype=np.float32).reshape(1, 1), (C, 1))
    )
    return [
        {
            "x": np.ascontiguousarray(
                x[i * NS:(i + 1) * NS].reshape(NS, CIN, HW)
            ).astype(bf16_np),
            "fc_weights": wT,
            "gama": g64,
        }
        for i in range(NCORES)
    ]


def kernel(x: np.ndarray, fc_weights: np.ndarray, gama: np.ndarray) -> np.ndarray:
    from concourse.bass_utils import run_bass_kernel_spmd

    nc = _get_nc()
    in_maps = _make_in_maps(x, fc_weights, gama)
    res = run_bass_kernel_spmd(nc, in_maps, core_ids=list(range(NCORES)))
    out = np.concatenate(
        [
            res.results[i]["out"].astype(np.float32).reshape(NS, CIN, H, W)
            for i in range(NCORES)
        ],
        axis=0,
    )
    return out
